# revision 7
# baseline (speedup 1.0000x reference)
"""
CSAM (channel self-attention) Trainium2 Bass kernel.

Computation (per batch b):
    q = x[b].reshape(C, N)                 # C=64, N=192*192=36864
    E = q @ q.T                            # [64, 64] channel gram
    A = softmax(rowmax(E) - E) over rows   # == softmax(-E) stabilized by rowmin
    out = A @ q
    res = x * (gamma * out) + x

Sharding: data-parallel over batch. 8 cores x 4 batches each; each core runs an
identical NEFF on its own batch slice (no collectives).

Per-core layout ("stacked"): q is held in SBUF as [128, 18432] fp32 where
partition p = 64*h + c holds q[c, 18432*h + j] (h in {0,1} halves of N).
 - energy: qT tiles come from an SBUF->SBUF xbar DMA transpose of the bf16 cast
   (qT[p, t, c'] = q_bf16[c', 128*t + p]); 288 accumulating bf16 matmuls of
   [128,64]^T @ [128,64] into one PSUM bank.
 - softmax: row-min (DVE) + exp on ACT with fused accum row-sum; A scaled by
   reciprocal; A^T via two PE transposes into both PSUM partition halves.
 - out: per 512-col chunk, two quadrant-packed bf16 matmuls (contraction = 64
   channels at partitions 0-63 / 64-127) -> PSUM [128, 512].
 - epilogue: ACT g = gamma*out + 1 (from PSUM), DVE res = g * x, DMA store.
"""

import os
import sys

sys.path.insert(0, "/opt/trn_rl_repo")

import numpy as np

import concourse.bass as bass
import concourse.bacc as bacc
import concourse.tile as tile
from concourse import mybir
from concourse.bass_utils import run_bass_kernel_spmd
from concourse.masks import make_identity

N_CORES = 8
B_FULL, C, H, W = 32, 64, 192, 192
N = H * W                 # 36864
NH = N // 2               # 18432 stacked free size
B_PER = B_FULL // N_CORES  # 4 batches per core
NQ = 4                    # load quarters per batch
QW = NH // NQ             # 4608 quarter width
TPQ = QW // 128           # 36 transpose chunks per quarter
CHUNK = 512               # out-matmul free dim (one PSUM bank)
CPB = NH // CHUNK         # 36 chunks per batch
RES_W = 2048              # store tile width (4 chunks)

f32 = mybir.dt.float32
bf16 = mybir.dt.bfloat16

_CACHED_NC = None


def _build():
    nc = bacc.Bacc("TRN2", target_bir_lowering=False, debug=False)
    x_d = nc.dram_tensor("x", [B_PER, C, N], f32, kind="ExternalInput").ap()
    g_d = nc.dram_tensor("gamma", [1], f32, kind="ExternalInput").ap()
    o_d = nc.dram_tensor("out", [B_PER, C, N], f32, kind="ExternalOutput").ap()

    with tile.TileContext(nc) as tc:
        with (
            tc.tile_pool(name="const", bufs=1) as constp,
            tc.tile_pool(name="qf", bufs=5) as qfp,
            tc.tile_pool(name="qb", bufs=4) as qbp,
            tc.tile_pool(name="qT", bufs=1) as qtp,
            tc.tile_pool(name="res", bufs=2) as resp,
            tc.tile_pool(name="g", bufs=3) as gp,
            tc.tile_pool(name="sm", bufs=2) as smp,
            tc.tile_pool(name="psE", bufs=2, space="PSUM") as psE,
            tc.tile_pool(name="psO", bufs=3, space="PSUM") as psO,
            tc.tile_pool(name="psT", bufs=2, space="PSUM") as psT,
        ):
            ident = constp.tile([128, 128], bf16)
            make_identity(nc, ident[:])
            g1 = constp.tile([1, 1], f32)
            nc.sync.dma_start(g1[:], g_d[None, :])
            gb = constp.tile([128, 1], f32)
            nc.gpsimd.partition_broadcast(gb[:], g1[:])

            for b in range(B_PER):
                xb = x_d[b].rearrange("c (h j) -> h c j", h=2)  # [2, 64, 18432]
                ob = o_d[b].rearrange("c (h j) -> h c j", h=2)

                # ---- phase 1: load, cast, transpose, energy ----
                qf = []
                qb = []
                qT = qtp.tile([128, NH // 128, 128], bf16, tag="qT")
                E = psE.tile([C, C], f32, tag="E")
                for k in range(NQ):
                    qfk = qfp.tile([128, QW], f32, tag="qf")
                    nc.sync.dma_start(
                        qfk[:], xb[:, :, k * QW : (k + 1) * QW]
                    )
                    qbk = qbp.tile([128, QW], bf16, tag="qb")
                    # split the cast between DVE and ACT
                    if k % 2 == 0:
                        nc.vector.tensor_copy(qbk[:], qfk[:])
                    else:
                        nc.scalar.copy(qbk[:], qfk[:])
                    nc.scalar.dma_start_transpose(
                        qT[:, k * TPQ : (k + 1) * TPQ, :], qbk[:]
                    )
                    qf.append(qfk)
                    qb.append(qbk)
                    for tt in range(TPQ):
                        t = k * TPQ + tt
                        for h in range(2):
                            nc.tensor.matmul(
                                E[:],
                                qT[:, t, 64 * h : 64 * h + 64],
                                qT[:, t, 64 * h : 64 * h + 64],
                                start=(t == 0 and h == 0),
                                stop=(t == NH // 128 - 1 and h == 1),
                            )

                # ---- softmax over rows of -E (row-min stabilized) ----
                m = smp.tile([C, 1], f32, tag="m")
                nc.vector.tensor_reduce(
                    m[:], E[:], axis=mybir.AxisListType.X, op=mybir.AluOpType.min
                )
                texp = smp.tile([C, C], f32, tag="texp")
                Z = smp.tile([C, 1], f32, tag="Z")
                nc.scalar.activation(
                    texp[:],
                    E[:],
                    mybir.ActivationFunctionType.Exp,
                    bias=m[:],
                    scale=-1.0,
                    accum_out=Z[:],
                )
                r = smp.tile([C, 1], f32, tag="r")
                nc.vector.reciprocal(r[:], Z[:])
                A = smp.tile([C, C], bf16, tag="A")
                nc.vector.tensor_scalar_mul(A[:], texp[:], r[:])

                # A^T into both partition halves of PSUM, then to SBUF bf16
                ATp = psT.tile([128, C], bf16, tag="ATp")
                nc.tensor.transpose(ATp[0:64, :], A[:], ident[0:64, 0:64])
                nc.tensor.transpose(
                    ATp[64:128, :], A[:], ident[0:64, 0:64], tile_position=(0, 64)
                )
                AT = smp.tile([128, C], bf16, tag="AT")
                nc.scalar.copy(AT[0:64, :], ATp[0:64, :])
                nc.scalar.copy(AT[64:128, :], ATp[64:128, :])

                # ---- phase 2: out = A @ q, epilogue, store ----
                for jj in range(CPB // 4):
                    res = resp.tile([128, RES_W], f32, tag="res")
                    for i in range(4):
                        j = jj * 4 + i
                        k, off = divmod(j * CHUNK, QW)
                        po = psO.tile([128, CHUNK], f32, tag="po")
                        nc.tensor.matmul(
                            po[0:64, :],
                            AT[0:64, :],
                            qb[k][0:64, off : off + CHUNK],
                            start=True,
                            stop=True,
                        )
                        nc.tensor.matmul(
                            po[64:128, :],
                            AT[64:128, :],
                            qb[k][64:128, off : off + CHUNK],
                            start=True,
                            stop=True,
                            tile_position=(64, 64),
                        )
                        gt = gp.tile([128, CHUNK], f32, tag="g")
                        nc.scalar.activation(
                            gt[:],
                            po[:],
                            mybir.ActivationFunctionType.Copy,
                            bias=1.0,
                            scale=gb[:],
                        )
                        nc.vector.tensor_tensor(
                            res[:, i * CHUNK : (i + 1) * CHUNK],
                            gt[:],
                            qf[k][:, off : off + CHUNK],
                            mybir.AluOpType.mult,
                        )
                    nc.sync.dma_start(
                        ob[:, :, jj * RES_W : (jj + 1) * RES_W], res[:]
                    )

    nc.compile()
    return nc


def _get_nc():
    global _CACHED_NC
    if _CACHED_NC is None:
        _CACHED_NC = _build()
    return _CACHED_NC


def kernel(x: np.ndarray, gamma: np.ndarray, _collect=None) -> np.ndarray:
    assert x.shape == (B_FULL, C, H, W) and x.dtype == np.float32
    nc = _get_nc()
    xr = np.ascontiguousarray(x.reshape(B_FULL, C, N), dtype=np.float32)
    gamma = np.ascontiguousarray(gamma, dtype=np.float32)
    in_maps = [
        {"x": xr[i * B_PER : (i + 1) * B_PER], "gamma": gamma}
        for i in range(N_CORES)
    ]
    r = run_bass_kernel_spmd(nc, in_maps, core_ids=list(range(N_CORES)))
    if _collect is not None:
        _collect.append(r)
    out = np.concatenate([r.results[i]["out"] for i in range(N_CORES)], axis=0)
    return out.reshape(B_FULL, C, H, W).astype(np.float32)


# revision 26
# speedup vs baseline: 12042.4005x; 12042.4005x over previous
"""
CSAM (channel self-attention) Trainium2 Bass kernel.

Computation (per batch b):
    q = x[b].reshape(C, N)                 # C=64, N=192*192=36864
    E = q @ q.T                            # [64, 64] channel gram
    A = softmax(rowmax(E) - E) over rows   # == softmax(-E) stabilized by rowmin
    out = A @ q
    res = x * (gamma * out) + x

Sharding: data-parallel over batch. 8 cores x 4 batches each; each core runs an
identical NEFF on its own batch slice (no collectives).

Per-core layout ("stacked"): q is held in SBUF as [128, 18432] fp32 where
partition p = 64*h + c holds q[c, 18432*h + j] (h in {0,1} halves of N).
 - energy: qT tiles come from an SBUF->SBUF xbar DMA transpose of the bf16 cast
   (qT[p, t, c'] = q_bf16[c', 128*t + p]); 288 accumulating bf16 matmuls of
   [128,64]^T @ [128,64] into one PSUM bank.
 - softmax: row-min (DVE) + exp on ACT with fused accum row-sum; A scaled by
   reciprocal; A^T via two PE transposes into both PSUM partition halves.
 - out: per 512-col chunk, two quadrant-packed bf16 matmuls (contraction = 64
   channels at partitions 0-63 / 64-127) -> PSUM [128, 512].
 - epilogue: ACT g = gamma*out + 1 (from PSUM), DVE res = g * x, DMA store.
"""

import os
import sys

sys.path.insert(0, "/opt/trn_rl_repo")

import numpy as np

import concourse.bass as bass
import concourse.bacc as bacc
import concourse.tile as tile
from concourse import mybir
from concourse.bass_utils import run_bass_kernel_spmd
from concourse.masks import make_identity

N_CORES = 8
B_FULL, C, H, W = 32, 64, 192, 192
N = H * W                 # 36864
NH = N // 2               # 18432 stacked free size
B_PER = B_FULL // N_CORES  # 4 batches per core
NQ = 4                    # load quarters per batch
QW = NH // NQ             # 4608 quarter width
TPQ = QW // 128           # 36 transpose chunks per quarter
CHUNK = 512               # out-matmul free dim (one PSUM bank)
CPB = NH // CHUNK         # 36 chunks per batch
RES_W = 2048              # store tile width (4 chunks)

f32 = mybir.dt.float32
bf16 = mybir.dt.bfloat16

_CACHED_NC = None
_STAGES = os.environ.get("KBENCH_STAGES", "full")
# chunks per quarter transposed via xbar DMA (rest go through the PE)
DMA_CHUNKS = int(os.environ.get("KBENCH_DMA_CHUNKS", "0"))
_NOSTORE = os.environ.get("KBENCH_NOSTORE", "") == "1"
_NOEPI = os.environ.get("KBENCH_NOEPI", "") == "1"
_REPS = int(os.environ.get("KBENCH_REPS", "1"))


def _build():
    nc = bacc.Bacc("TRN2", target_bir_lowering=False, debug=False)
    x_d = nc.dram_tensor("x", [B_PER, C, N], f32, kind="ExternalInput").ap()
    g_d = nc.dram_tensor("gamma", [1], f32, kind="ExternalInput").ap()
    o_d = nc.dram_tensor("out", [B_PER, C, N], f32, kind="ExternalOutput").ap()

    with tile.TileContext(nc) as tc:
        with (
            tc.tile_pool(name="const", bufs=1) as constp,
            tc.tile_pool(name="qf", bufs=5) as qfp,
            tc.tile_pool(name="qb", bufs=5) as qbp,
            tc.tile_pool(name="qT", bufs=1) as qtp,
            tc.tile_pool(name="res", bufs=2) as resp,
            tc.tile_pool(name="sm", bufs=2) as smp,
            tc.tile_pool(name="psE", bufs=1, space="PSUM") as psE,
            tc.tile_pool(name="psO", bufs=3, space="PSUM") as psO,
            tc.tile_pool(name="psT", bufs=2, space="PSUM") as psT,
            tc.tile_pool(name="psA", bufs=1, space="PSUM") as psA,
        ):
            identf = constp.tile([128, 128], f32)
            make_identity(nc, identf[:])
            g1 = constp.tile([1, 1], f32)
            nc.sync.dma_start(g1[:], g_d[None, :])
            gb = constp.tile([128, 1], f32)
            nc.gpsimd.partition_broadcast(gb[:], g1[:])
            identb = constp.tile([64, 64], bf16)
            make_identity(nc, identb[:])

            qf_holder = [None]
            qb_holder = [None]
            E2_holder = [None]

            def phase1(b):
                xb = x_d[b].rearrange("c (h j) -> h c j", h=2)  # [2, 64, 18432]
                qf = []
                qb = []
                qf_holder[0] = qf
                qb_holder[0] = qb
                qT = qtp.tile([128, NH // 128, 128], bf16, tag="qT")
                # paired-energy PSUM: block [0:64,0:64] accumulates the
                # h=0 half of E, [64:128,64:128] the h=1 half (off-diagonal
                # blocks are cross-half junk, never read)
                E2a = psE.tile([C, C], f32, tag="E0")
                E2b = psE.tile([C, C], f32, tag="E1")
                E2 = (E2a, E2b)
                E2_holder[0] = E2
                D = DMA_CHUNKS
                for k in range(NQ):
                    qfk = qfp.tile([128, QW], f32, tag="qf")
                    nc.sync.dma_start(qfk[:], xb[:, :, k * QW : (k + 1) * QW])
                    qbk = qbp.tile([128, QW], bf16, tag="qb")
                    if D > 0:
                        nc.scalar.copy(qbk[:, : D * 128], qfk[:, : D * 128])
                        nc.scalar.dma_start_transpose(
                            qT[:, k * TPQ : k * TPQ + D, :], qbk[:, : D * 128]
                        )
                        nc.scalar.copy(qbk[:, D * 128 :], qfk[:, D * 128 :])
                    else:
                        nc.scalar.copy(qbk[:], qfk[:])
                    qf.append(qfk)
                    qb.append(qbk)
                    # remaining chunks: one [128,128] fp32 PE transpose each,
                    # 4 chunks per PSUM bank, one batched ACT copy per group
                    assert (TPQ - D) % 4 == 0
                    for tt0 in range(D, TPQ, 4):
                        pq = psT.tile([128, 4, 128], f32, tag="pq")
                        for ti in range(4):
                            tt = tt0 + ti
                            nc.tensor.transpose(
                                pq[:, ti, :],
                                qfk[:, tt * 128 : (tt + 1) * 128],
                                identf[:],
                            )
                        nc.scalar.copy(
                            qT[:, k * TPQ + tt0 : k * TPQ + tt0 + 4, :], pq[:]
                        )
                    if _STAGES in ("loads", "trans"):
                        continue
                    for tt in range(TPQ):
                        t = k * TPQ + tt
                        for h in range(2):
                            nc.tensor.matmul(
                                E2[h][:],
                                qT[:, t, 64 * h : 64 * h + 64],
                                qT[:, t, 64 * h : 64 * h + 64],
                                start=(t == 0),
                                stop=(t == NH // 128 - 1),
                            )
                    yield k

            def softmax_pre(E2):
                # E = h0 block + h1 block (only one PSUM operand allowed)
                E1s = smp.tile([C, C], f32, tag="E1s")
                nc.scalar.copy(E1s[:], E2[1][:])
                E = smp.tile([C, C], f32, tag="E")
                nc.vector.tensor_tensor(
                    E[:], E2[0][:], E1s[:], mybir.AluOpType.add
                )
                # row-min-stabilized softmax of -E (no PE work here)
                m = smp.tile([C, 1], f32, tag="m")
                nc.vector.tensor_reduce(
                    m[:], E[:], axis=mybir.AxisListType.X, op=mybir.AluOpType.min
                )
                texp = smp.tile([C, C], f32, tag="texp")
                Z = smp.tile([C, 1], f32, tag="Z")
                nc.scalar.activation(
                    texp[:],
                    E[:],
                    mybir.ActivationFunctionType.Exp,
                    bias=m[:],
                    scale=-1.0,
                    accum_out=Z[:],
                )
                r = smp.tile([C, 1], f32, tag="r")
                nc.vector.reciprocal(r[:], Z[:])
                # fold gamma into A so the epilogue is res = (out + 1) * x
                rg = smp.tile([C, 1], f32, tag="rg")
                nc.vector.tensor_tensor(rg[:], r[:], gb[0:64, :], mybir.AluOpType.mult)
                A = smp.tile([C, C], bf16, tag="A")
                nc.vector.tensor_scalar_mul(A[:], texp[:], rg[:])
                return A

            def make_AT(A):
                # A^T into both partition halves of PSUM, then to SBUF bf16
                ATp = psA.tile([128, C], bf16, tag="ATp")
                nc.tensor.transpose(ATp[0:64, :], A[:], identb[:])
                nc.tensor.transpose(
                    ATp[64:128, :], A[:], identb[:], tile_position=(0, 64)
                )
                AT = smp.tile([128, C], bf16, tag="AT")
                nc.scalar.copy(AT[0:64, :], ATp[0:64, :])
                nc.scalar.copy(AT[64:128, :], ATp[64:128, :])
                return AT

            def phase2_group(b, qf, qb, AT, jj):
                ob = o_d[b].rearrange("c (h j) -> h c j", h=2)
                if True:
                    res = resp.tile([128, RES_W], f32, tag="res")
                    for i in range(4):
                        j = jj * 4 + i
                        k, off = divmod(j * CHUNK, QW)
                        po = psO.tile([128, CHUNK], f32, tag="po")
                        nc.tensor.matmul(
                            po[0:64, :],
                            AT[0:64, :],
                            qb[k][0:64, off : off + CHUNK],
                            start=True,
                            stop=True,
                        )
                        nc.tensor.matmul(
                            po[64:128, :],
                            AT[64:128, :],
                            qb[k][64:128, off : off + CHUNK],
                            start=True,
                            stop=True,
                            tile_position=(64, 64),
                        )
                        if _NOEPI:
                            continue
                        nc.vector.scalar_tensor_tensor(
                            res[:, i * CHUNK : (i + 1) * CHUNK],
                            po[:],
                            1.0,
                            qf[k][:, off : off + CHUNK],
                            mybir.AluOpType.add,
                            mybir.AluOpType.mult,
                        )
                    if not (_NOSTORE or _NOEPI):
                        nc.sync.dma_start(
                            ob[:, :, jj * RES_W : (jj + 1) * RES_W], res[:]
                        )

            def phase2(b, qf, qb, AT):
                for jj in range(CPB // 4):
                    phase2_group(b, qf, qb, AT, jj)

            # software pipeline, interleaved at quarter granularity: while
            # batch b loads/casts/transposes/accumulates, batch b-1's
            # out-matmul+epilogue groups drain in between
            prev = None
            for b in [bb % B_PER for bb in range(B_PER * _REPS)]:
                it1 = phase1(b)
                if _STAGES in ("loads", "trans", "energy"):
                    for _ in it1:
                        pass
                    continue
                ng = CPB // 4
                for ki, _ in enumerate(it1):
                    if prev is not None and _STAGES == "full":
                        for jj in range(ki * 2, min(ng, ki * 2 + 2)):
                            phase2_group(*prev, jj)
                A = softmax_pre(E2_holder[0])
                if prev is not None and _STAGES == "full":
                    for jj in range(NQ * 2, ng):
                        phase2_group(*prev, jj)
                AT = make_AT(A)
                prev = (b, qf_holder[0], qb_holder[0], AT)
            if _STAGES == "full":
                phase2(*prev)

    nc.compile()
    return nc


def _get_nc():
    global _CACHED_NC
    if _CACHED_NC is None:
        _CACHED_NC = _build()
    return _CACHED_NC


def kernel(x: np.ndarray, gamma: np.ndarray, _collect=None) -> np.ndarray:
    assert x.shape == (B_FULL, C, H, W) and x.dtype == np.float32
    nc = _get_nc()
    xr = np.ascontiguousarray(x.reshape(B_FULL, C, N), dtype=np.float32)
    gamma = np.ascontiguousarray(gamma, dtype=np.float32)
    in_maps = [
        {"x": xr[i * B_PER : (i + 1) * B_PER], "gamma": gamma}
        for i in range(N_CORES)
    ]
    r = run_bass_kernel_spmd(nc, in_maps, core_ids=list(range(N_CORES)))
    if _collect is not None:
        _collect.append(r)
    out = np.concatenate([r.results[i]["out"] for i in range(N_CORES)], axis=0)
    return out.reshape(B_FULL, C, H, W).astype(np.float32)
